# revision 2
# baseline (speedup 1.0000x reference)
"""Co-attention kernel for Trainium2, 8-core data-parallel over batch.

reference math (per batch):
  a  = q @ d.T                      [Lq, Ld]
  aq = softmax_q(mask_q(a))         (softmax over dim q)
  ad = softmax_d(mask_d(a.T))       (softmax over dim d)
  sd = q.T @ aq                     [H, Ld]
  sq = d.T @ ad                     [H, Lq]
  cd = sq @ aq                      [H, Ld]
  returns (cd.T, sq.T, sd.T)        ([Ld,H], [Lq,H], [Ld,H])

On-chip layout strategy (per batch, per core):
  Qn [q,h] f32r  --PE transpose--> QT [h,q] f32r
  Dn [d,h] f32r  --PE transpose--> DT [h,d] f32r
  AT [d,q] = DT.T @ QT (f32r matmul, psum->sbuf f32)
  A  [q,d] = transpose(AT) fused with +maskd on copy-out
  softmax-q on AT rows (free dim = q): +maskq, rowmax, exp(+sum), scale by 1/sum
    -> EqT [d,q] == aq.T ; transpose -> Eq [q,d] f32r  (Eq == aq, normalized)
  softmax-d on A rows (free dim = d): rowmax, exp(+sum), scale
    -> EdT [q,d] == ad.T ; transpose -> EdL [d,q] f32r (EdL == ad, normalized)
  sdT [d,h] = Eq.T @ Qn    (lhsT=Eq k-tiles over q, rhs=Qn)   -> out sd.T
  sqT [q,h] = EdL.T @ Dn   (lhsT=EdL k-tiles over d, rhs=Dn)  -> out sq.T
  cdT [d,h] = Eq.T @ sqT   (lhsT=Eq, rhs=sqT f32r)            -> out cd.T
"""

import hashlib
import os
import shutil
import tempfile
from pathlib import Path

import numpy as np

B, L, H = 32, 1024, 1024  # Lq == Ld == H == 1024
NCORES = 8
BPC = B // NCORES  # batches per core
NT = L // 128      # 8 row-tiles per matrix
NEG = -1e9

_NEFF_CACHE = os.environ.get(
    "NEFF_CACHE_DIR", os.path.join(tempfile.gettempdir(), "neff_cache")
)


def _install_neff_cache():
    import concourse.bass2jax as b2j

    orig = b2j.compile_bir_kernel
    if getattr(b2j, "_neff_cache_installed", False):
        return
    os.makedirs(_NEFF_CACHE, exist_ok=True)

    def cached(bir_json, tmpdir, neff_name="file.neff"):
        if isinstance(bir_json, str):
            bir_json = bir_json.encode()
        key = hashlib.sha256(bir_json).hexdigest()
        hit = Path(_NEFF_CACHE) / f"{key}.neff"
        out = Path(tmpdir) / neff_name
        if hit.exists():
            shutil.copyfile(hit, out)
            return str(out)
        res = orig(bir_json, tmpdir, neff_name)
        try:
            shutil.copyfile(res, hit)
        except OSError:
            pass
        return res

    b2j.compile_bir_kernel = cached
    b2j._neff_cache_installed = True


def build_module(bpc=BPC, reps=1):
    """Build + compile the per-core Bass module. Returns the Bacc object."""
    import concourse.bacc as bacc
    import concourse.bass as bass
    import concourse.tile as tile
    from concourse import mybir
    from concourse.masks import make_identity

    f32 = mybir.dt.float32
    f32r = mybir.dt.float32r
    i32 = mybir.dt.int32

    nc = bacc.Bacc("TRN2", target_bir_lowering=False, debug=False)

    q_d = nc.dram_tensor("q", [bpc, L, H], f32r, kind="ExternalInput")
    d_d = nc.dram_tensor("d", [bpc, L, H], f32r, kind="ExternalInput")
    qlen_d = nc.dram_tensor("qlen", [bpc], f32, kind="ExternalInput")
    dlen_d = nc.dram_tensor("dlen", [bpc], f32, kind="ExternalInput")
    cd_d = nc.dram_tensor("cd", [bpc, L, H], f32, kind="ExternalOutput")
    sq_d = nc.dram_tensor("sq", [bpc, L, H], f32, kind="ExternalOutput")
    sd_d = nc.dram_tensor("sd", [bpc, L, H], f32, kind="ExternalOutput")

    with tile.TileContext(nc) as tc:
        _build_body(nc, tc, bass, mybir, make_identity,
                    q_d, d_d, qlen_d, dlen_d, cd_d, sq_d, sd_d, bpc, reps)

    nc.compile()
    return nc


def _build_body(nc, tc, bass, mybir, make_identity,
                q_d, d_d, qlen_d, dlen_d, cd_d, sq_d, sd_d, bpc, reps):
    from contextlib import ExitStack

    f32 = mybir.dt.float32
    f32r = mybir.dt.float32r
    i32 = mybir.dt.int32

    with ExitStack() as ctx:
        const = ctx.enter_context(tc.tile_pool(name="const", bufs=1))
        big = ctx.enter_context(tc.tile_pool(name="big", bufs=36))
        stage = ctx.enter_context(tc.tile_pool(name="stage", bufs=4))
        maskp = ctx.enter_context(tc.tile_pool(name="maskp", bufs=4))
        small = ctx.enter_context(tc.tile_pool(name="small", bufs=48))
        pmm = ctx.enter_context(tc.tile_pool(name="pmm", bufs=4, space="PSUM"))
        ptr = ctx.enter_context(tc.tile_pool(name="ptr", bufs=4, space="PSUM"))

        # --- constants -------------------------------------------------
        ident = const.tile([128, 128], f32)
        make_identity(nc, ident)
        ident_r = const.tile([128, 128], f32r)
        nc.vector.tensor_copy(ident_r, ident)
        iota_i = const.tile([128, L], i32)
        nc.gpsimd.iota(iota_i, pattern=[[1, L]], base=0, channel_multiplier=0)
        iota_f = const.tile([128, L], f32)
        nc.vector.tensor_copy(iota_f, iota_i)

        def mat(name):
            # allocate one [1024, 1024] matrix as 8 tiles [128, 1024]
            return [big.tile([128, L], f32, name=f"{name}_{r}", tag="mat")
                    for r in range(NT)]

        def mat_r(name):
            return [big.tile([128, L], f32r, name=f"{name}_{r}", tag="mat")
                    for r in range(NT)]

        def load_mat(dst, dram, b):
            for r in range(NT):
                nc.sync.dma_start(
                    out=dst[r], in_=dram.ap()[b, 128 * r:128 * (r + 1), :])

        def pe_transpose(src, dst_dtype_r, name, fuse_add=None, out_dt=None):
            """dst = src.T (8x8 grid of 128x128 PE transposes).

            src: list of 8 tiles [128, L]; returns new mat tiles.
            fuse_add: optional [128, L] mask tile added during copy-out (DVE).
            dst_dtype_r: True -> dst tiles f32r (copy-out converts).
            """
            dst = mat_r(name) if dst_dtype_r else mat(name)
            src_r = src[0].dtype == f32r
            idn = ident_r if src_r else ident
            pdt = f32r if src_r else f32
            for r2 in range(NT):
                for cg in range(2):  # two 512-wide column groups
                    pst = ptr.tile([128, 512], pdt, name=f"pst_{name}", tag="pst")
                    for cc in range(4):
                        c = 4 * cg + cc
                        nc.tensor.transpose(
                            pst[:, 128 * cc:128 * (cc + 1)],
                            src[c][:, 128 * r2:128 * (r2 + 1)],
                            idn)
                    out_sl = dst[r2][:, 512 * cg:512 * (cg + 1)]
                    if fuse_add is not None:
                        nc.vector.tensor_add(
                            out_sl, pst, fuse_add[:, 512 * cg:512 * (cg + 1)])
                    else:
                        nc.scalar.copy(out=out_sl, in_=pst)
            return dst

        def emit_mm(lhsT, rhs, consume, name):
            """out[m,n] = sum_k lhsT[k][:,m-block] . rhs[k][:,n-strip].

            lhsT: 8 k-tiles [128, L(m)]; rhs: 8 k-tiles [128, L(n)].
            consume(r, ns, psum_ap) for each (m-tile r, 512-strip ns).
            """
            for r in range(NT):
                for ns in range(2):
                    ps = pmm.tile([128, 512], f32, name=f"ps_{name}", tag="ps")
                    for k in range(NT):
                        nc.tensor.matmul(
                            ps,
                            lhsT[k][:, 128 * r:128 * (r + 1)],
                            rhs[k][:, 512 * ns:512 * (ns + 1)],
                            start=(k == 0), stop=(k == NT - 1))
                    consume(r, ns, ps)

        def softmax_rows(src, name):
            """Masked rows already; softmax along free dim of each tile.

            Returns normalized exp tiles (f32) — src is consumed (read only).
            """
            out = mat(name)
            for r in range(NT):
                mx = small.tile([128, 1], f32, name=f"mx_{name}", tag="mx")
                nc.vector.reduce_max(mx, src[r], axis=mybir.AxisListType.X)
                nmx = small.tile([128, 1], f32, name=f"nmx_{name}", tag="nmx")
                nc.vector.tensor_scalar_mul(nmx, mx, -1.0)
                sm = small.tile([128, 1], f32, name=f"sm_{name}", tag="sm")
                nc.scalar.activation(
                    out=out[r], in_=src[r],
                    func=mybir.ActivationFunctionType.Exp,
                    bias=nmx, scale=1.0, accum_out=sm)
                inv = small.tile([128, 1], f32, name=f"inv_{name}", tag="inv")
                nc.vector.reciprocal(inv, sm)
                nc.vector.tensor_scalar_mul(out[r], out[r], inv)
            return out

        def bcast_len(dram, b, name):
            t = small.tile([128, 1], f32, name=name, tag=name)
            src = bass.AP(tensor=dram, offset=b, ap=[[0, 128], [1, 1]])
            nc.sync.dma_start(out=t, in_=src)
            return t

        for _rep in range(reps):
            for b in range(bpc):
                # --- load + masks -----------------------------------------
                qlen = bcast_len(qlen_d, b, "qlen_t")
                dlen = bcast_len(dlen_d, b, "dlen_t")
                maskq = maskp.tile([128, L], f32, name="maskq", tag="mk")
                nc.vector.tensor_scalar(
                    out=maskq, in0=iota_f, scalar1=qlen, scalar2=NEG,
                    op0=mybir.AluOpType.is_ge, op1=mybir.AluOpType.mult)
                maskd = maskp.tile([128, L], f32, name="maskd", tag="mk")
                nc.vector.tensor_scalar(
                    out=maskd, in0=iota_f, scalar1=dlen, scalar2=NEG,
                    op0=mybir.AluOpType.is_ge, op1=mybir.AluOpType.mult)

                Qn = mat_r("Qn")
                load_mat(Qn, q_d, b)
                QT = pe_transpose(Qn, True, "QT")
                Dn = mat_r("Dn")
                load_mat(Dn, d_d, b)
                DT = pe_transpose(Dn, True, "DT")

                # --- AT = DT.T @ QT  ([d, q]) -----------------------------
                AT = mat("AT")

                def at_consume(r, ns, ps):
                    nc.scalar.copy(
                        out=AT[r][:, 512 * ns:512 * (ns + 1)], in_=ps)

                emit_mm(DT, QT, at_consume, "at")

                # --- A = AT.T + maskd  ([q, d]) ---------------------------
                A = pe_transpose(AT, False, "A", fuse_add=maskd)

                # --- softmax over q (on AT rows) --------------------------
                for r in range(NT):
                    nc.vector.tensor_add(AT[r], AT[r], maskq)
                EqT = softmax_rows(AT, "EqT")         # [d, q] = aq.T
                Eq = pe_transpose(EqT, True, "Eq")    # [q, d] = aq (f32r)

                # --- softmax over d (on A rows) ---------------------------
                EdT = softmax_rows(A, "EdT")          # [q, d] = ad.T
                EdL = pe_transpose(EdT, True, "EdL")  # [d, q] = ad (f32r)

                # --- sdT = Eq.T @ Qn  ([d, h]) ----------------------------
                Qn2 = mat_r("Qn2")
                load_mat(Qn2, q_d, b)
                sd_stage = {}

                def sd_consume(r, ns, ps):
                    if r not in sd_stage:
                        sd_stage[r] = stage.tile(
                            [128, L], f32, name="sd_st", tag="st")
                    st = sd_stage[r]
                    nc.scalar.copy(out=st[:, 512 * ns:512 * (ns + 1)], in_=ps)
                    if ns == 1:
                        nc.sync.dma_start(
                            out=sd_d.ap()[b, 128 * r:128 * (r + 1), :], in_=st)

                emit_mm(Eq, Qn2, sd_consume, "sd")

                # --- sqT = EdL.T @ Dn  ([q, h]) ---------------------------
                Dn2 = mat_r("Dn2")
                load_mat(Dn2, d_d, b)
                sqT = mat_r("sqT")

                def sq_consume(r, ns, ps):
                    nc.scalar.copy(
                        out=sqT[r][:, 512 * ns:512 * (ns + 1)], in_=ps)
                    if ns == 1:
                        nc.sync.dma_start(
                            out=sq_d.ap()[b, 128 * r:128 * (r + 1), :],
                            in_=sqT[r].bitcast(mybir.dt.float32))

                emit_mm(EdL, Dn2, sq_consume, "sq")

                # --- cdT = Eq.T @ sqT  ([d, h]) ---------------------------
                cd_stage = {}

                def cd_consume(r, ns, ps):
                    if r not in cd_stage:
                        cd_stage[r] = stage.tile(
                            [128, L], f32, name="cd_st", tag="st")
                    st = cd_stage[r]
                    nc.scalar.copy(out=st[:, 512 * ns:512 * (ns + 1)], in_=ps)
                    if ns == 1:
                        nc.sync.dma_start(
                            out=cd_d.ap()[b, 128 * r:128 * (r + 1), :], in_=st)

                emit_mm(Eq, sqT, cd_consume, "cd")


_MODULE = None


def _get_module():
    global _MODULE
    if _MODULE is None:
        _install_neff_cache()
        _MODULE = build_module()
    return _MODULE


def kernel(q, d, q_len, d_len):
    from concourse.bass_utils import run_bass_kernel_spmd

    nc = _get_module()
    q = np.ascontiguousarray(q, dtype=np.float32)
    d = np.ascontiguousarray(d, dtype=np.float32)
    qlen_f = np.ascontiguousarray(q_len).astype(np.float32)
    dlen_f = np.ascontiguousarray(d_len).astype(np.float32)

    in_maps = []
    for c in range(NCORES):
        s = slice(c * BPC, (c + 1) * BPC)
        in_maps.append({
            "q": q[s], "d": d[s], "qlen": qlen_f[s], "dlen": dlen_f[s]})

    res = run_bass_kernel_spmd(nc, in_maps, core_ids=list(range(NCORES)))
    cd = np.concatenate([res.results[c]["cd"] for c in range(NCORES)], axis=0)
    sq = np.concatenate([res.results[c]["sq"] for c in range(NCORES)], axis=0)
    sd = np.concatenate([res.results[c]["sd"] for c in range(NCORES)], axis=0)
    return cd, sq, sd
